# revision 1
# baseline (speedup 1.0000x reference)
"""DGCNN forward on 8 Trainium2 NeuronCores (Bass/Tile), data-parallel over batch.

Per core (one point cloud, N=1024, k=20):
  EdgeConv h[o,i,j] = W @ [x_i ; x_j - x_i] decomposes as a[o,i] + b[o,j]
  (a = (W1-W2) @ Y, b = W2 @ Y), and since BN scale gamma=1 > 0 the max over
  neighbors commutes with BN+ReLU:
      y[o,i] = relu(s_o * (a[o,i] + max_{j in N(i)} b[o,j]) + t_o)
  knn needs only per-row ranking, so pd[i,j] = 2<x_i,x_j> - |x_j|^2.
  Top-20 per row via DVE max8 / max_index / match_replace (3 rounds).
  Neighbor max via indirect-DMA gather of bT rows + strided DVE max-reduce.
  Batch-wide BN statistics via mask identities (M[i,j] = pd[i,j] >= v20[i]):
      sum h   = k*sum_i a_i        + sum_j deg_j b_j
      sum h^2 = k*sum_i a_i^2 + 2*<b, M^T a> + sum_j deg_j b_j^2
  where M^T-contractions run on the tensor engine (M as lhsT contracts its
  partition dim = i), followed by an 8-core AllReduce of the partial sums.
"""

import sys

for _p in ("/opt/trn_rl_repo", "/root/.axon_site/_ro/trn_rl_repo"):
    if _p not in sys.path:
        sys.path.insert(0, _p)

from contextlib import ExitStack

import numpy as np

import concourse.bass as bass
import concourse.mybir as mybir
from concourse.bass import IndirectOffsetOnAxis
from concourse.masks import make_identity
from concourse.tile import TileContext

f32 = mybir.dt.float32
u32 = mybir.dt.uint32
AF = mybir.ActivationFunctionType
OP = mybir.AluOpType

P = 128
N = 1024
NT = N // P
KNN = 20
B = 8
NCORES = 8
LAYERS = [(3, 64), (64, 64), (64, 128), (128, 256)]
CF = 512
COF = 1024
EPS = 1e-5
NEGINF = -1.0e30
BNK = float(B * N * KNN)
BN_ = float(B * N)


def _split_multi_waits(nc, max_waits=1):
    """The walrus build here rejects >1 sync-wait per instruction; splitting
    extras onto prepended single-wait drains on the same engine is
    semantically identical (the sequencer blocks on each wait in turn)."""
    n = 0
    for func in nc.m.functions:
        for bb in func.blocks:
            out = []
            for ins in bb.instructions:
                si = getattr(ins, "sync_info", None)
                waits = list(si.on_wait) if si is not None and si.on_wait else []
                if len(waits) > max_waits:
                    extra, keep = waits[:-max_waits], waits[-max_waits:]
                    for i, w in enumerate(extra):
                        out.append(
                            mybir.InstDrain(
                                name=f"{ins.name}-ws{i}",
                                engine=ins.engine,
                                ins=[],
                                outs=[],
                                sync_info=mybir.SyncInfo(on_wait=[w], on_update=[]),
                            )
                        )
                        n += 1
                    si.on_wait = keep
                out.append(ins)
            bb.instructions[:] = out
    return n


def _w5_slice(w5k, kofs, cw, mt):
    kc, ko = kofs // P, kofs % P
    assert ko + cw <= P
    return w5k[kc][ko:ko + cw, mt * P:(mt + 1) * P]


def _build():
    nc = bass.Bass()

    x_in = nc.dram_tensor("x", [3, N], f32, kind="ExternalInput")
    wdt_d, w2t_d, g_d, bb_d = [], [], [], []
    for li, (C, Co) in enumerate(LAYERS):
        wdt_d.append(nc.dram_tensor(f"wdt{li}", [C, Co], f32, kind="ExternalInput"))
        w2t_d.append(nc.dram_tensor(f"w2t{li}", [C, Co], f32, kind="ExternalInput"))
        g_d.append(nc.dram_tensor(f"g{li}", [Co, 1], f32, kind="ExternalInput"))
        bb_d.append(nc.dram_tensor(f"bb{li}", [Co, 1], f32, kind="ExternalInput"))
    w5t_d = nc.dram_tensor("w5t", [CF, COF], f32, kind="ExternalInput")
    g5_d = nc.dram_tensor("g5", [COF, 1], f32, kind="ExternalInput")
    b5_d = nc.dram_tensor("b5", [COF, 1], f32, kind="ExternalInput")
    out_d = nc.dram_tensor("out", [COF, 1], f32, kind="ExternalOutput")

    groups = [list(range(NCORES))]

    with TileContext(nc) as tc, ExitStack() as ctx:
        sb = ctx.enter_context(tc.tile_pool(name="sb", bufs=1))
        wk = ctx.enter_context(tc.tile_pool(name="wk", bufs=2))
        ps = ctx.enter_context(tc.tile_pool(name="ps", bufs=2, space="PSUM"))
        dr = ctx.enter_context(tc.tile_pool(name="dr", bufs=1, space="DRAM"))

        ident = sb.tile([P, P], f32, name="ident")
        make_identity(nc, ident[:])
        ones_col = sb.tile([P, 1], f32, name="ones_col")
        nc.vector.memset(ones_col[:], 1.0)
        ones_row = sb.tile([1, P], f32, name="ones_row")
        nc.vector.memset(ones_row[:], 1.0)
        neg_col = sb.tile([P, 1], f32, name="neg_col")
        nc.vector.memset(neg_col[:], -1.0)

        Y = sb.tile([3, N], f32, name="Y0")
        nc.sync.dma_start(out=Y[:], in_=x_in[:])
        Y = Y[:, :]

        w5k = []
        for kc in range(CF // P):
            t_ = sb.tile([P, COF], f32, name=f"w5k{kc}")
            nc.sync.dma_start(out=t_[:], in_=w5t_d[kc * P:(kc + 1) * P, :])
            w5k.append(t_)
        g5c, b5c = [], []
        for mt in range(COF // P):
            tg = sb.tile([P, 1], f32, name=f"g5c{mt}")
            tb = sb.tile([P, 1], f32, name=f"b5c{mt}")
            nc.sync.dma_start(out=tg[:], in_=g5_d[mt * P:(mt + 1) * P, :])
            nc.sync.dma_start(out=tb[:], in_=b5_d[mt * P:(mt + 1) * P, :])
            g5c.append(tg)
            b5c.append(tb)

        pd_sb = [sb.tile([P, N], f32, name=f"pd{t}") for t in range(NT)]
        idx_sb = [sb.tile([P, 24], u32, name=f"idx{t}") for t in range(NT)]
        v24_sb = [sb.tile([P, 24], f32, name=f"v24{t}") for t in range(NT)]
        aT_sb = [sb.tile([P, 257], f32, name=f"aT{t}") for t in range(NT)]
        bT_sb = [sb.tile([P, 256], f32, name=f"bT{t}") for t in range(NT)]

        # K-chunks for the final conv: PE needs lhsT/rhs on the same base
        # partition, so the 64-channel outputs of L1 and L2 are packed into
        # one 128-row tile (rows 0:64 and 64:128).
        catk0 = sb.tile([P, N], f32, name="catk0")
        cat_pieces = [(catk0, P)]

        for li, (C, Co) in enumerate(LAYERS):
            mc_n = (Co + P - 1) // P

            wdt = wk.tile([C, Co], f32, tag="wdt", name=f"wdt_s{li}")
            w2t = wk.tile([C, Co], f32, tag="w2t", name=f"w2t_s{li}")
            nc.sync.dma_start(out=wdt[:], in_=wdt_d[li][:])
            nc.sync.dma_start(out=w2t[:], in_=w2t_d[li][:])
            g_col = wk.tile([P, mc_n], f32, tag="g_col", name=f"g_s{li}")
            b_col = wk.tile([P, mc_n], f32, tag="b_col", name=f"b_s{li}")
            nc.vector.memset(g_col[:], 1.0)
            nc.vector.memset(b_col[:], 0.0)
            for mc in range(mc_n):
                cw = min(P, Co - mc * P)
                nc.sync.dma_start(
                    out=g_col[:cw, mc:mc + 1], in_=g_d[li][mc * P:mc * P + cw, :]
                )
                nc.sync.dma_start(
                    out=b_col[:cw, mc:mc + 1], in_=bb_d[li][mc * P:mc * P + cw, :]
                )

            # ---- norms row: x2n = -sum_c Y^2 ----
            sq = wk.tile([C, N], f32, tag="sq", bufs=1, name=f"sq{li}")
            nc.scalar.square(out=sq[:], in_=Y)
            Y2 = wk.tile([C, N], f32, tag="Y2", bufs=1, name=f"Y2_{li}")
            nc.scalar.mul(out=Y2[:], in_=Y, mul=2.0)
            x2n = wk.tile([1, N], f32, tag="x2n", bufs=1, name=f"x2n{li}")
            for nh in range(2):
                x2ps = ps.tile([1, 512], f32, tag="acc", bufs=1, name=f"x2ps{li}_{nh}")
                nc.tensor.matmul(
                    out=x2ps[:],
                    lhsT=neg_col[:C, :],
                    rhs=sq[:, nh * 512:(nh + 1) * 512],
                    start=True,
                    stop=True,
                )
                nc.scalar.copy(out=x2n[:, nh * 512:(nh + 1) * 512], in_=x2ps[:])

            # ---- aT, bT per i-tile; bT also to DRAM for the gather ----
            bTd = dr.tile([N, Co], f32, name=f"bTd{li}")
            for t in range(NT):
                yc = Y[:, t * P:(t + 1) * P]
                aps = ps.tile([P, 257], f32, tag="mm", name=f"aTps{li}_{t}")
                nc.tensor.matmul(
                    out=aps[:, :Co], lhsT=yc, rhs=wdt[:], start=True, stop=True
                )
                nc.scalar.copy(out=aT_sb[t][:, :Co], in_=aps[:, :Co])
                nc.vector.memset(aT_sb[t][:, Co:Co + 1], 1.0)
                bps = ps.tile([P, 257], f32, tag="mm", name=f"bTps{li}_{t}")
                nc.tensor.matmul(
                    out=bps[:, :Co], lhsT=yc, rhs=w2t[:], start=True, stop=True
                )
                nc.scalar.copy(out=bT_sb[t][:, :Co], in_=bps[:, :Co])
                nc.sync.dma_start(
                    out=bTd[t * P:(t + 1) * P, :], in_=bT_sb[t][:, :Co]
                )

            # ---- a (o-part, i-free) ----
            a_sb = [
                wk.tile([P, N], f32, tag=f"a{mc}", name=f"a{li}_{mc}")
                for mc in range(mc_n)
            ]
            for mc in range(mc_n):
                cw = min(P, Co - mc * P)
                for nh in range(2):
                    aps2 = ps.tile([P, 512], f32, tag="mm", name=f"aps{li}_{mc}_{nh}")
                    nc.tensor.matmul(
                        out=aps2[:cw, :],
                        lhsT=wdt[:, mc * P:mc * P + cw],
                        rhs=Y[:, nh * 512:(nh + 1) * 512],
                        start=True,
                        stop=True,
                    )
                    nc.scalar.copy(
                        out=a_sb[mc][:cw, nh * 512:(nh + 1) * 512], in_=aps2[:cw, :]
                    )

            # ---- per i-tile: pd, topk, mask, gather, k-max, m^T ----
            mT_sb = [
                wk.tile([P, N], f32, tag=f"mT{mc}", name=f"mT{li}_{mc}")
                for mc in range(mc_n)
            ]
            for t in range(NT):
                pdps = ps.tile([P, N], f32, tag="pd", name=f"pdps{li}_{t}")
                for nh in range(2):
                    o = pdps[:, nh * 512:(nh + 1) * 512]
                    nc.tensor.matmul(
                        out=o,
                        lhsT=Y2[:, t * P:(t + 1) * P],
                        rhs=Y[:, nh * 512:(nh + 1) * 512],
                        start=True,
                        stop=False,
                    )
                    nc.tensor.matmul(
                        out=o,
                        lhsT=ones_row[:],
                        rhs=x2n[:, nh * 512:(nh + 1) * 512],
                        start=False,
                        stop=True,
                    )
                pd = pd_sb[t]
                nc.scalar.copy(out=pd[:], in_=pdps[:])

                v24, idx = v24_sb[t], idx_sb[t]
                pm1 = wk.tile([P, N], f32, tag="pm", name=f"pm1_{li}_{t}")
                pm2 = wk.tile([P, N], f32, tag="pm", name=f"pm2_{li}_{t}")
                nc.vector.max(out=v24[:, 0:8], in_=pd[:])
                nc.vector.max_index(
                    out=idx[:, 0:8], in_max=v24[:, 0:8], in_values=pd[:]
                )
                nc.vector.match_replace(
                    out=pm1[:], in_to_replace=v24[:, 0:8], in_values=pd[:],
                    imm_value=NEGINF,
                )
                nc.vector.max(out=v24[:, 8:16], in_=pm1[:])
                nc.vector.max_index(
                    out=idx[:, 8:16], in_max=v24[:, 8:16], in_values=pm1[:]
                )
                nc.vector.match_replace(
                    out=pm2[:], in_to_replace=v24[:, 8:16], in_values=pm1[:],
                    imm_value=NEGINF,
                )
                nc.vector.max(out=v24[:, 16:24], in_=pm2[:])
                nc.vector.max_index(
                    out=idx[:, 16:24], in_max=v24[:, 16:24], in_values=pm2[:]
                )

                # mask in place: pd <- (pd >= v20)
                nc.vector.tensor_scalar(
                    out=pd[:], in0=pd[:], scalar1=v24[:, 19:20], scalar2=None,
                    op0=OP.is_ge,
                )

                # neighbor max: per-partition single-offset gathers (the
                # only indirect-DMA form this runtime supports) into per-slot
                # slices, then a strided DVE max-reduce over the slot axis
                nbr = wk.tile([P, KNN, Co], f32, tag="nbr", bufs=1,
                              name=f"nbr{li}_{t}")
                for s in range(KNN):
                    nc.gpsimd.indirect_dma_start(
                        out=nbr[:, s, :],
                        out_offset=None,
                        in_=bTd[:],
                        in_offset=IndirectOffsetOnAxis(
                            ap=idx[:, s:s + 1], axis=0
                        ),
                    )
                m_t = wk.tile([P, 256], f32, tag="mt", name=f"m{li}_{t}")
                nc.vector.tensor_reduce(
                    out=m_t[:, :Co],
                    in_=nbr[:, :, :].rearrange("p s c -> p c s"),
                    axis=mybir.AxisListType.X,
                    op=OP.max,
                )
                for mc in range(mc_n):
                    cw = min(P, Co - mc * P)
                    tp = ps.tile([P, P], f32, tag="mm", name=f"tp{li}_{t}_{mc}")
                    nc.tensor.transpose(
                        out=tp[:cw, :], in_=m_t[:, mc * P:mc * P + cw],
                        identity=ident[:],
                    )
                    nc.scalar.copy(
                        out=mT_sb[mc][:cw, t * P:(t + 1) * P], in_=tp[:cw, :]
                    )

            # ---- BN partial sums ----
            # PSUM accumulation groups must not interleave within one bank
            # (start zeroes the whole zero-region), so every ones-contraction
            # is a closed single-matmul group into its own column; the
            # per-iteration partials are then reduced on the DVE.
            # cols [jc*6 + arr*2 + mc] for the jc-loop arrays (crs/bsum/bsq),
            # cols [48 + t*4 + arr*2 + mc] for the a-side (asum/asq).
            acc = ps.tile([P, 80], f32, tag="acc", bufs=1, name=f"acc{li}")
            nc.vector.memset(acc[:], 0.0)

            def col_reduce(col, src_ap):
                for mc in range(mc_n):
                    cw = min(P, Co - mc * P)
                    nc.tensor.matmul(
                        out=acc[:cw, col + mc:col + mc + 1],
                        lhsT=src_ap[:, mc * P:mc * P + cw],
                        rhs=ones_col[:],
                        start=True,
                        stop=True,
                    )

            for jc in range(NT):
                gps = ps.tile([P, 257], f32, tag="mm", name=f"gps{li}_{jc}")
                for t in range(NT):
                    nc.tensor.matmul(
                        out=gps[:, :Co + 1],
                        lhsT=pd_sb[t][:, jc * P:(jc + 1) * P],
                        rhs=aT_sb[t][:, :Co + 1],
                        start=(t == 0),
                        stop=(t == NT - 1),
                    )
                G = wk.tile([P, 257], f32, tag="G", name=f"G{li}_{jc}")
                nc.scalar.copy(out=G[:, :Co + 1], in_=gps[:, :Co + 1])
                bt = bT_sb[jc][:, :Co]
                pr_c = wk.tile([P, 256], f32, tag="prc", name=f"prc{li}_{jc}")
                nc.vector.tensor_tensor(
                    out=pr_c[:, :Co], in0=bt, in1=G[:, :Co], op=OP.mult
                )
                pr_s = wk.tile([P, 256], f32, tag="prs", name=f"prs{li}_{jc}")
                nc.vector.tensor_scalar(
                    out=pr_s[:, :Co], in0=bt, scalar1=G[:, Co:Co + 1], scalar2=None,
                    op0=OP.mult,
                )
                bsq_t = wk.tile([P, 256], f32, tag="bsqt", name=f"bsqt{li}_{jc}")
                nc.scalar.square(out=bsq_t[:, :Co], in_=bt)
                pr_q = wk.tile([P, 256], f32, tag="prq", name=f"prq{li}_{jc}")
                nc.vector.tensor_scalar(
                    out=pr_q[:, :Co], in0=bsq_t[:, :Co], scalar1=G[:, Co:Co + 1],
                    scalar2=None, op0=OP.mult,
                )
                col_reduce(jc * 6 + 0, pr_c[:, :])
                col_reduce(jc * 6 + 2, pr_s[:, :])
                col_reduce(jc * 6 + 4, pr_q[:, :])
            for t in range(NT):
                aT2 = wk.tile([P, 256], f32, tag="aT2", name=f"aT2_{li}_{t}")
                nc.scalar.square(out=aT2[:, :Co], in_=aT_sb[t][:, :Co])
                col_reduce(48 + t * 4 + 0, aT_sb[t][:, :256])
                col_reduce(48 + t * 4 + 2, aT2[:, :])

            # columns: sh = k*asum + bsum ; sq = k*asq + (2*cross + bsq)
            accs = wk.tile([P, 16], f32, tag="accs", name=f"accs{li}")
            jview = acc[:, 0:48].rearrange("p (jc s) -> p s jc", s=6)
            tview = acc[:, 48:80].rearrange("p (t s) -> p s t", s=4)
            for arr in range(3):
                for mc in range(mc_n):
                    nc.vector.tensor_reduce(
                        out=accs[:, arr * 2 + mc:arr * 2 + mc + 1],
                        in_=jview[:, arr * 2 + mc, :],
                        axis=mybir.AxisListType.X,
                        op=OP.add,
                    )
            for arr in range(2):
                for mc in range(mc_n):
                    nc.vector.tensor_reduce(
                        out=accs[:, 6 + arr * 2 + mc:6 + arr * 2 + mc + 1],
                        in_=tview[:, arr * 2 + mc, :],
                        axis=mybir.AxisListType.X,
                        op=OP.add,
                    )
            stc = wk.tile([P, 4], f32, tag="stc", name=f"stc{li}")
            nc.vector.memset(stc[:], 0.0)
            nc.vector.scalar_tensor_tensor(
                out=stc[:, 0:mc_n], in0=accs[:, 6:6 + mc_n], scalar=float(KNN),
                in1=accs[:, 2:2 + mc_n], op0=OP.mult, op1=OP.add,
            )
            tmpc = wk.tile([P, 2], f32, tag="tmpc", name=f"tmpc{li}")
            nc.vector.scalar_tensor_tensor(
                out=tmpc[:, 0:mc_n], in0=accs[:, 0:mc_n], scalar=2.0,
                in1=accs[:, 4:4 + mc_n], op0=OP.mult, op1=OP.add,
            )
            nc.vector.scalar_tensor_tensor(
                out=stc[:, 2:2 + mc_n], in0=accs[:, 8:8 + mc_n], scalar=float(KNN),
                in1=tmpc[:, 0:mc_n], op0=OP.mult, op1=OP.add,
            )

            # ---- AllReduce across cores ----
            cc_in = dr.tile([P * 4], f32, name=f"ccin{li}")
            cc_out = dr.tile([P * 4], f32, name=f"ccout{li}", addr_space="Shared")
            nc.sync.dma_start(
                out=cc_in[:].rearrange("(p a) -> p a", p=P), in_=stc[:, :]
            )
            nc.gpsimd.collective_compute(
                "AllReduce", OP.add, replica_groups=groups,
                ins=[cc_in[:]], outs=[cc_out[:]],
            )
            stats = wk.tile([P, 4], f32, tag="stats", name=f"stats{li}")
            nc.sync.dma_start(
                out=stats[:, :], in_=cc_out[:].rearrange("(p a) -> p a", p=P)
            )

            # ---- s,t columns ----
            mu = wk.tile([P, 2], f32, tag="mu", name=f"mu{li}")
            nc.scalar.mul(out=mu[:, :mc_n], in_=stats[:, 0:mc_n], mul=1.0 / BNK)
            e2 = wk.tile([P, 2], f32, tag="e2", name=f"e2{li}")
            nc.scalar.mul(out=e2[:, :mc_n], in_=stats[:, 2:2 + mc_n], mul=1.0 / BNK)
            mu2 = wk.tile([P, 2], f32, tag="mu2", name=f"mu2_{li}")
            nc.scalar.square(out=mu2[:, :mc_n], in_=mu[:, :mc_n])
            veps = wk.tile([P, 2], f32, tag="veps", name=f"veps{li}")
            nc.vector.scalar_tensor_tensor(
                out=veps[:, :mc_n], in0=e2[:, :mc_n], scalar=EPS, in1=mu2[:, :mc_n],
                op0=OP.add, op1=OP.subtract,
            )
            sd = wk.tile([P, 2], f32, tag="sd", name=f"sd{li}")
            nc.scalar.sqrt(out=sd[:, :mc_n], in_=veps[:, :mc_n])
            rstd = wk.tile([P, 2], f32, tag="rstd", name=f"rstd{li}")
            nc.vector.reciprocal(out=rstd[:, :mc_n], in_=sd[:, :mc_n])
            s_col = wk.tile([P, 2], f32, tag="s_col", name=f"scol{li}")
            nc.vector.tensor_tensor(
                out=s_col[:, :mc_n], in0=g_col[:, :mc_n], in1=rstd[:, :mc_n],
                op=OP.mult,
            )
            mus = wk.tile([P, 2], f32, tag="mus", name=f"mus{li}")
            nc.vector.tensor_tensor(
                out=mus[:, :mc_n], in0=mu[:, :mc_n], in1=s_col[:, :mc_n], op=OP.mult
            )
            t_col = wk.tile([P, 2], f32, tag="t_col", name=f"tcol{li}")
            nc.vector.tensor_tensor(
                out=t_col[:, :mc_n], in0=b_col[:, :mc_n], in1=mus[:, :mc_n],
                op=OP.subtract,
            )

            # ---- y = relu(s*(a + mT) + t) ----
            y_tiles = []
            for mc in range(mc_n):
                cw = min(P, Co - mc * P)
                nc.vector.tensor_tensor(
                    out=a_sb[mc][:cw, :], in0=a_sb[mc][:cw, :],
                    in1=mT_sb[mc][:cw, :], op=OP.add,
                )
                yt = sb.tile([P, N], f32, name=f"Yout{li}_{mc}")
                nc.scalar.activation(
                    out=yt[:cw, :], in_=a_sb[mc][:cw, :], func=AF.Relu,
                    bias=t_col[:cw, mc:mc + 1], scale=s_col[:cw, mc:mc + 1],
                )
                y_tiles.append((yt, cw))
                if li < 2:
                    nc.scalar.copy(
                        out=catk0[li * 64:(li + 1) * 64, :], in_=yt[:cw, :]
                    )
                else:
                    cat_pieces.append((yt, cw))

            if li + 1 < len(LAYERS):
                assert mc_n == 1
                Y = y_tiles[0][0][:Co, :]

        # ---------------- final conv ----------------
        ccf_in = dr.tile([2 * COF], f32, name="ccf_in")
        ccf_out = dr.tile([2 * COF], f32, name="ccf_out", addr_space="Shared")

        def h5_matmuls(hps, mt):
            kofs = 0
            npieces = len(cat_pieces)
            for pi, (yt, cw) in enumerate(cat_pieces):
                for nh in range(2):
                    nc.tensor.matmul(
                        out=hps[:, nh * 512:(nh + 1) * 512],
                        lhsT=_w5_slice(w5k, kofs, cw, mt),
                        rhs=yt[:cw, nh * 512:(nh + 1) * 512],
                        start=(pi == 0),
                        stop=(pi == npieces - 1),
                    )
                kofs += cw

        # pass 1: statistics only
        for mt in range(COF // P):
            hps = ps.tile([P, N], f32, tag="pd", name=f"h5ps{mt}")
            h5_matmuls(hps, mt)
            h5scr = wk.tile([P, N], f32, tag="h5scr", name=f"h5scr{mt}")
            sum_c = wk.tile([P, 1], f32, tag="sum_c", name=f"sumc{mt}")
            nc.scalar.activation(
                out=h5scr[:], in_=hps[:], func=AF.Copy, accum_out=sum_c[:]
            )
            sqs_c = wk.tile([P, 1], f32, tag="sqs_c", name=f"sqsc{mt}")
            nc.scalar.activation(
                out=h5scr[:], in_=h5scr[:], func=AF.Square, accum_out=sqs_c[:]
            )
            nc.sync.dma_start(
                out=ccf_in[mt * P:(mt + 1) * P],
                in_=sum_c[:].rearrange("p o -> (p o)"),
            )
            nc.sync.dma_start(
                out=ccf_in[COF + mt * P:COF + (mt + 1) * P],
                in_=sqs_c[:].rearrange("p o -> (p o)"),
            )
        nc.gpsimd.collective_compute(
            "AllReduce", OP.add, replica_groups=groups,
            ins=[ccf_in[:]], outs=[ccf_out[:]],
        )
        # pass 2: recompute h5, fused BN+ReLU on evacuation, then max over N
        for mt in range(COF // P):
            shc = wk.tile([P, 1], f32, tag="shc5", name=f"shc5_{mt}")
            sqc = wk.tile([P, 1], f32, tag="sqc5", name=f"sqc5_{mt}")
            nc.sync.dma_start(out=shc[:], in_=ccf_out[mt * P:(mt + 1) * P, None])
            nc.sync.dma_start(
                out=sqc[:], in_=ccf_out[COF + mt * P:COF + (mt + 1) * P, None]
            )
            mu = wk.tile([P, 1], f32, tag="mu5", name=f"mu5_{mt}")
            nc.scalar.mul(out=mu[:], in_=shc[:], mul=1.0 / BN_)
            e2 = wk.tile([P, 1], f32, tag="e25", name=f"e25_{mt}")
            nc.scalar.mul(out=e2[:], in_=sqc[:], mul=1.0 / BN_)
            mu2 = wk.tile([P, 1], f32, tag="mu25", name=f"mu25_{mt}")
            nc.scalar.square(out=mu2[:], in_=mu[:])
            veps = wk.tile([P, 1], f32, tag="veps5", name=f"veps5_{mt}")
            nc.vector.scalar_tensor_tensor(
                out=veps[:], in0=e2[:], scalar=EPS, in1=mu2[:],
                op0=OP.add, op1=OP.subtract,
            )
            sd = wk.tile([P, 1], f32, tag="sd5", name=f"sd5_{mt}")
            nc.scalar.sqrt(out=sd[:], in_=veps[:])
            rstd = wk.tile([P, 1], f32, tag="rstd5", name=f"rstd5_{mt}")
            nc.vector.reciprocal(out=rstd[:], in_=sd[:])
            s_c = wk.tile([P, 1], f32, tag="s_c5", name=f"sc5_{mt}")
            nc.vector.tensor_tensor(
                out=s_c[:], in0=g5c[mt][:], in1=rstd[:], op=OP.mult
            )
            mus = wk.tile([P, 1], f32, tag="mus5", name=f"mus5_{mt}")
            nc.vector.tensor_tensor(out=mus[:], in0=mu[:], in1=s_c[:], op=OP.mult)
            t_c = wk.tile([P, 1], f32, tag="t_c5", name=f"tc5_{mt}")
            nc.vector.tensor_tensor(
                out=t_c[:], in0=b5c[mt][:], in1=mus[:], op=OP.subtract
            )
            hps = ps.tile([P, N], f32, tag="pd", name=f"h5ps2_{mt}")
            h5_matmuls(hps, mt)
            y5 = wk.tile([P, N], f32, tag="h5scr", name=f"y5_{mt}")
            nc.scalar.activation(
                out=y5[:], in_=hps[:], func=AF.Relu, bias=t_c[:], scale=s_c[:]
            )
            rmax = wk.tile([P, 1], f32, tag="rmax", name=f"rmax{mt}")
            nc.vector.reduce_max(out=rmax[:], in_=y5[:], axis=mybir.AxisListType.X)
            nc.sync.dma_start(out=out_d[mt * P:(mt + 1) * P, :], in_=rmax[:])

    _split_multi_waits(nc)
    return nc


_NC_CACHE = {}


def kernel(x, w1, g1, b1, w2, g2, b2, w3, g3, b3, w4, g4, b4, w5, g5, b5):
    from concourse.bass_utils import run_bass_kernel_spmd

    if "nc" not in _NC_CACHE:
        _NC_CACHE["nc"] = _build()
    nc = _NC_CACHE["nc"]

    ws = [np.asarray(w1), np.asarray(w2), np.asarray(w3), np.asarray(w4)]
    gs = [np.asarray(g1), np.asarray(g2), np.asarray(g3), np.asarray(g4)]
    bs = [np.asarray(b1), np.asarray(b2), np.asarray(b3), np.asarray(b4)]
    x = np.asarray(x, dtype=np.float32)

    base = {}
    for li, w in enumerate(ws):
        Co, twoC = w.shape
        C = twoC // 2
        W1, W2 = w[:, :C], w[:, C:]
        base[f"wdt{li}"] = np.ascontiguousarray((W1 - W2).T, dtype=np.float32)
        base[f"w2t{li}"] = np.ascontiguousarray(W2.T, dtype=np.float32)
        base[f"g{li}"] = np.ascontiguousarray(
            gs[li].reshape(Co, 1), dtype=np.float32
        )
        base[f"bb{li}"] = np.ascontiguousarray(
            bs[li].reshape(Co, 1), dtype=np.float32
        )
    base["w5t"] = np.ascontiguousarray(np.asarray(w5).T, dtype=np.float32)
    base["g5"] = np.ascontiguousarray(
        np.asarray(g5).reshape(COF, 1), dtype=np.float32
    )
    base["b5"] = np.ascontiguousarray(
        np.asarray(b5).reshape(COF, 1), dtype=np.float32
    )

    in_maps = []
    for c in range(NCORES):
        m = dict(base)
        m["x"] = np.ascontiguousarray(x[c], dtype=np.float32)
        in_maps.append(m)

    res = run_bass_kernel_spmd(nc, in_maps, list(range(NCORES)))
    out = np.stack([res.results[c]["out"].reshape(COF) for c in range(NCORES)])
    return out.astype(np.float32)

